# revision 4
# baseline (speedup 1.0000x reference)
"""Multi-head self-attention (B=2, S=2048, D=1024, H=16) on 8 Trainium2 cores.

Sharding: Megatron-style tensor parallelism on the head dimension.
Each core owns 2 heads (128 of the 1024 model dims):
  - Wq/Wk/Wv column-sharded: core c computes Q/K/V for dims [c*128,(c+1)*128)
  - attention for its 2 heads over both batches
  - Wo row-sharded: core c produces a partial output [4096, 1024]
  - host sums the 8 partials and adds bo.

Per-core device layouts (matmuls run as float32r = full-rate PE with
~tf32 multiply precision, fp32 accumulate):
  qT/kT: [128(out-dim), 4096(token)]  "o-major"
  v:     token-major k-tiles [128(token), 132] = 2x [head(64) | ones | pad]
         (the ones column makes the PV matmul also produce the softmax
          normalizer as output row 64; pad keeps the stationary free dim even,
          a float32r requirement)
  scores are computed transposed: sT[k, q] = (kT tile).T @ qT chunk, so the
  softmax sum reduces over the PARTITION dim -- done for free by the ones row
  in the PV matmul instead of a vector reduction. exp() needs no max
  subtraction: scores*0.125 are ~N(0,1) for this problem family, far from
  fp32 overflow.
"""

import numpy as np
from contextlib import ExitStack

import concourse.bass as bass
import concourse.tile as tile
from concourse import bacc, mybir
from concourse.bass_utils import run_bass_kernel_spmd
from concourse.masks import make_identity

B, S, D = 2, 2048, 1024
H, DH = 16, 64
T = B * S                  # 4096 tokens total
N_CORES = 8
OPC = D // N_CORES         # 128 out dims per core
HPC = H // N_CORES         # 2 heads per core
NI = D // 128              # 8 contraction chunks of 128
TCH = 512                  # projection token chunk
NTCH = T // TCH            # 8
QCH = 512                  # attention q chunk
NQCH = S // QCH            # 4 per batch
NKT = S // 128             # 16 key tiles per batch
HW = DH + 2                # 66 cols per head in the v tile (data|ones|pad)
VW = HPC * HW              # 132

F32 = mybir.dt.float32
F32R = mybir.dt.float32r
EXP = mybir.ActivationFunctionType.Exp


def _mha_kernel(ctx: ExitStack, tc, y, xT, wq, wk, wv, woT, bq, bk, bv):
    nc = tc.nc
    pers = ctx.enter_context(tc.tile_pool(name="pers", bufs=1))

    qT = pers.tile([128, T], F32R, tag="qT")
    kT = pers.tile([128, T], F32R, tag="kT")
    vT = pers.tile([128, T], F32, tag="vT")
    vtk = pers.tile([128, B * NKT, VW], F32R, tag="vtk")
    wq_sb = pers.tile([128, NI, OPC], F32R, tag="wq")
    wk_sb = pers.tile([128, NI, OPC], F32R, tag="wk")
    wv_sb = pers.tile([128, NI, OPC], F32R, tag="wv")
    woT_sb = pers.tile([128, D], F32R, tag="wo")
    bq_sb = pers.tile([128, 1], F32, tag="bq")
    bk_sb = pers.tile([128, 1], F32, tag="bk")
    bv_sb = pers.tile([128, 1], F32, tag="bv")
    ident = pers.tile([128, 128], F32, tag="ident")

    nc.sync.dma_start(wq_sb, wq)
    nc.sync.dma_start(wk_sb, wk)
    nc.sync.dma_start(wv_sb, wv)
    nc.sync.dma_start(woT_sb, woT)
    nc.sync.dma_start(bq_sb, bq)
    nc.sync.dma_start(bk_sb, bk)
    nc.sync.dma_start(bv_sb, bv)
    make_identity(nc, ident)
    # constant ones/pad columns of vtk (memset can't write float32r directly)
    onepad = pers.tile([128, 2], F32, tag="onepad")
    nc.vector.memset(onepad[:, 0:1], 1.0)
    nc.vector.memset(onepad[:, 1:2], 0.0)
    onepad_b = bass.AP(
        tensor=onepad.tensor,
        offset=onepad.offset,
        ap=[onepad.ap[0], [0, B * NKT], onepad.ap[1]],
    )
    for h in range(HPC):
        nc.vector.tensor_copy(
            vtk[:, :, h * HW + DH : h * HW + DH + 2], onepad_b
        )

    # Phase A: Q/K/V projections in o-major layout, then transpose V to
    # token-major k-tiles.
    with (
        tc.tile_pool(name="psA", bufs=2, space="PSUM") as psA,
        tc.tile_pool(name="xin", bufs=4) as xin,
    ):
        for t in range(NTCH):
            ps_q = psA.tile([128, TCH], F32, tag="ps_q")
            ps_k = psA.tile([128, TCH], F32, tag="ps_k")
            ps_v = psA.tile([128, TCH], F32, tag="ps_v")
            for i in range(NI):
                xt = xin.tile([128, TCH], F32R, tag="xt")
                nc.sync.dma_start(xt, xT[i, :, t * TCH : (t + 1) * TCH])
                st, sp = (i == 0), (i == NI - 1)
                nc.tensor.matmul(ps_q, wq_sb[:, i, :], xt, start=st, stop=sp)
                nc.tensor.matmul(ps_k, wk_sb[:, i, :], xt, start=st, stop=sp)
                nc.tensor.matmul(ps_v, wv_sb[:, i, :], xt, start=st, stop=sp)
            sl = slice(t * TCH, (t + 1) * TCH)
            nc.vector.tensor_scalar_add(qT[:, sl], ps_q, bq_sb)
            nc.vector.tensor_scalar_add(kT[:, sl], ps_k, bk_sb)
            nc.vector.tensor_scalar_add(vT[:, sl], ps_v, bv_sb)

        for g in range(B * NKT):
            ps_t = psA.tile([128, 128], F32, tag="ps_t")
            nc.tensor.transpose(ps_t, vT[:, g * 128 : (g + 1) * 128], ident)
            for h in range(HPC):
                nc.vector.tensor_copy(
                    vtk[:, g, h * HW : h * HW + DH],
                    ps_t[:, h * DH : (h + 1) * DH],
                )

    # Phase B: attention + output projection.
    with (
        tc.tile_pool(name="psB", bufs=2, space="PSUM") as psB,
        tc.tile_pool(name="att", bufs=8) as att,
        tc.tile_pool(name="sm", bufs=3) as sm,
        tc.tile_pool(name="yo_p", bufs=3) as yo_p,
    ):
        for b in range(B):
            for qc in range(NQCH):
                q0 = b * S + qc * QCH
                ctx_sb = sm.tile([128, QCH], F32R, tag="ctx")
                for h in range(HPC):
                    hs = slice(h * DH, (h + 1) * DH)
                    ps_pv = psB.tile([HW, QCH], F32, tag="ps_pv")
                    at_tiles = []
                    for kt in range(NKT):
                        g = b * NKT + kt
                        ps_s = psB.tile([128, QCH], F32, tag="ps_s")
                        nc.tensor.matmul(
                            ps_s,
                            kT[hs, g * 128 : (g + 1) * 128],
                            qT[hs, q0 : q0 + QCH],
                            start=True,
                            stop=True,
                        )
                        at = att.tile([128, QCH], F32R, tag="at")
                        nc.scalar.activation(at, ps_s, EXP, scale=0.125)
                        at_tiles.append(at)
                    for kt in range(NKT):
                        g = b * NKT + kt
                        nc.tensor.matmul(
                            ps_pv,
                            vtk[:, g, h * HW : (h + 1) * HW],
                            at_tiles[kt],
                            start=(kt == 0),
                            stop=(kt == NKT - 1),
                        )
                    # normalize: ctx rows for this head = pv[0:64] * recip(pv[64])
                    rrow = sm.tile([1, QCH], F32, tag="rrow")
                    nc.vector.reciprocal(rrow, ps_pv[DH : DH + 1, :])
                    nrm = sm.tile([DH, QCH], F32, tag="nrm")
                    nc.gpsimd.partition_broadcast(nrm, rrow)
                    nc.vector.tensor_mul(
                        ctx_sb[h * DH : (h + 1) * DH, :], ps_pv[0:DH, :], nrm
                    )
                for t4 in range(QCH // 128):
                    yo = yo_p.tile([128, D], F32, tag="yo")
                    for nch in range(D // 512):
                        ps_o = psB.tile([128, 512], F32, tag="ps_o")
                        nc.tensor.matmul(
                            ps_o,
                            ctx_sb[:, t4 * 128 : (t4 + 1) * 128],
                            woT_sb[:, nch * 512 : (nch + 1) * 512],
                            start=True,
                            stop=True,
                        )
                        nc.vector.tensor_copy(yo[:, nch * 512 : (nch + 1) * 512], ps_o)
                    r0 = q0 + t4 * 128
                    nc.sync.dma_start(y[r0 : r0 + 128, :], yo)


_NC_CACHE = {}


def _build_nc():
    if "nc" in _NC_CACHE:
        return _NC_CACHE["nc"]
    nc = bacc.Bacc("TRN2", target_bir_lowering=False, debug=False, num_devices=N_CORES)
    xT = nc.dram_tensor("xT", [NI, 128, T], F32R, kind="ExternalInput").ap()
    wq = nc.dram_tensor("wq", [128, NI, OPC], F32R, kind="ExternalInput").ap()
    wk = nc.dram_tensor("wk", [128, NI, OPC], F32R, kind="ExternalInput").ap()
    wv = nc.dram_tensor("wv", [128, NI, OPC], F32R, kind="ExternalInput").ap()
    woT = nc.dram_tensor("woT", [128, D], F32R, kind="ExternalInput").ap()
    bq = nc.dram_tensor("bq", [128, 1], F32, kind="ExternalInput").ap()
    bk = nc.dram_tensor("bk", [128, 1], F32, kind="ExternalInput").ap()
    bv = nc.dram_tensor("bv", [128, 1], F32, kind="ExternalInput").ap()
    y = nc.dram_tensor("y", [T, D], F32, kind="ExternalOutput").ap()
    with tile.TileContext(nc) as tc:
        with ExitStack() as ctx:
            _mha_kernel(ctx, tc, y, xT, wq, wk, wv, woT, bq, bk, bv)
    nc.compile()
    _NC_CACHE["nc"] = nc
    return nc


def _prep_in_maps(inputs):
    x = np.asarray(inputs["x"], np.float32)
    Wq = np.asarray(inputs["Wq"], np.float32)
    Wk = np.asarray(inputs["Wk"], np.float32)
    Wv = np.asarray(inputs["Wv"], np.float32)
    Wo = np.asarray(inputs["Wo"], np.float32)
    bq = np.asarray(inputs["bq"], np.float32)
    bk = np.asarray(inputs["bk"], np.float32)
    bv = np.asarray(inputs["bv"], np.float32)

    xT_np = np.ascontiguousarray(x.reshape(T, D).T).reshape(NI, 128, T)

    def _w_slice(W, c):
        # [128(p), NI, OPC]: [p, i, o] = W[c*OPC+o, i*128+p]
        A = np.ascontiguousarray(W[c * OPC : (c + 1) * OPC, :].T)  # [D, OPC]
        return np.ascontiguousarray(A.reshape(NI, 128, OPC).transpose(1, 0, 2))

    in_maps = []
    for c in range(N_CORES):
        sl = slice(c * OPC, (c + 1) * OPC)
        in_maps.append(
            {
                "xT": xT_np,
                "wq": _w_slice(Wq, c),
                "wk": _w_slice(Wk, c),
                "wv": _w_slice(Wv, c),
                "woT": np.ascontiguousarray(Wo[:, sl].T),
                "bq": bq[sl].reshape(OPC, 1).copy(),
                "bk": bk[sl].reshape(OPC, 1).copy(),
                "bv": bv[sl].reshape(OPC, 1).copy(),
            }
        )
    return in_maps


def kernel(**inputs) -> np.ndarray:
    nc = _build_nc()
    in_maps = _prep_in_maps(inputs)
    res = run_bass_kernel_spmd(nc, in_maps, core_ids=list(range(N_CORES)))
    bo = np.asarray(inputs["bo"], np.float32)
    y = np.zeros((T, D), np.float64)
    for c in range(N_CORES):
        y += res.results[c]["y"].astype(np.float64)
    y = (y + bo).astype(np.float32)
    return y.reshape(B, S, D)


# revision 22
# speedup vs baseline: 30.7869x; 30.7869x over previous
"""Multi-head self-attention (B=2, S=2048, D=1024, H=16) on 8 Trainium2 cores.

Sharding: Megatron-style tensor parallelism on the head dimension.
Each core owns 2 heads (128 of the 1024 model dims):
  - Wq/Wk/Wv column-sharded: core c computes Q/K/V for dims [c*128,(c+1)*128)
  - attention for its 2 heads over both batches
  - Wo row-sharded: core c produces a partial output [4096, 1024]
  - host sums the 8 partials and adds bo.

Per-core device layouts (matmuls run as float32r = full-rate PE with
~tf32 multiply precision, fp32 accumulate):
  qT/kT: [128(out-dim), 4096(token)]  "o-major"
  v:     token-major k-tiles [128(token), 132] = 2x [head(64) | ones | pad]
         (the ones column makes the PV matmul also produce the softmax
          normalizer as output row 64; pad keeps the stationary free dim even,
          a float32r requirement)
  scores are computed transposed: sT[k, q] = (kT tile).T @ qT chunk, so the
  softmax sum reduces over the PARTITION dim -- done for free by the ones row
  in the PV matmul instead of a vector reduction. exp() needs no max
  subtraction: scores*0.125 are ~N(0,1) for this problem family, far from
  fp32 overflow.
"""

import os
import numpy as np
import ml_dtypes
from contextlib import ExitStack

import concourse.bass as bass
import concourse.tile as tile
from concourse import bacc, mybir
from concourse.bass_utils import run_bass_kernel_spmd
from concourse.masks import make_identity

B, S, D = 2, 2048, 1024
H, DH = 16, 64
T = B * S                  # 4096 tokens total
N_CORES = 8
OPC = D // N_CORES         # 128 out dims per core
HPC = H // N_CORES         # 2 heads per core
NI = D // 128              # 8 contraction chunks of 128
TCH = 512                  # projection token chunk
NTCH = T // TCH            # 8
QCH = 512                  # attention q chunk
NQCH = S // QCH            # 4 per batch
NKT = S // 128             # 16 key tiles per batch
HW = DH + 2                # 66 cols per head in the v tile (data|ones|pad)
VW = HPC * HW              # 132

F32 = mybir.dt.float32
F32R = mybir.dt.float32r
BF16 = mybir.dt.bfloat16
EXP = mybir.ActivationFunctionType.Exp

# matmul operand dtype: "f32r" (~tf32 precision, 2 PE cycles/row) or
# "bf16" (1 PE cycle/row + fast weight load, ~bf16 precision)
MM_MODE = os.environ.get("MHA_MM_DT", "f32r")
if MM_MODE == "bf16":
    MM_DT, MM_NP = BF16, ml_dtypes.bfloat16
else:
    MM_DT, MM_NP = F32R, np.float32
# dtype of the pre-transpose V staging and the transpose identity
VT_DT = BF16 if MM_MODE == "bf16" else F32


def _mha_kernel(tc, y, xT, wq, wk, wv, woT, bq, bk, bv):
    with ExitStack() as ctx:
        _mha_kernel_inner(ctx, tc, y, xT, wq, wk, wv, woT, bq, bk, bv)


def _mha_kernel_inner(ctx: ExitStack, tc, y, xT, wq, wk, wv, woT, bq, bk, bv):
    nc = tc.nc
    pers = ctx.enter_context(tc.tile_pool(name="pers", bufs=1))

    qT = pers.tile([128, T], MM_DT, tag="qT")
    kT = pers.tile([128, T], MM_DT, tag="kT")
    vT = pers.tile([128, T], VT_DT, tag="vT")
    vtk = pers.tile([128, B * NKT, VW], MM_DT, tag="vtk")
    wq_sb = pers.tile([128, NI, OPC], MM_DT, tag="wq")
    wk_sb = pers.tile([128, NI, OPC], MM_DT, tag="wk")
    wv_sb = pers.tile([128, NI, OPC], MM_DT, tag="wv")
    woT_sb = pers.tile([128, D], MM_DT, tag="wo")
    bq_sb = pers.tile([128, 1], F32, tag="bq")
    bk_sb = pers.tile([128, 1], F32, tag="bk")
    bv_sb = pers.tile([128, 1], F32, tag="bv")
    ident = pers.tile([128, 128], VT_DT, tag="ident")

    nc.sync.dma_start(wq_sb, wq)
    nc.sync.dma_start(wk_sb, wk)
    nc.sync.dma_start(wv_sb, wv)
    nc.sync.dma_start(woT_sb, woT)
    nc.sync.dma_start(bq_sb, bq)
    nc.sync.dma_start(bk_sb, bk)
    nc.sync.dma_start(bv_sb, bv)
    make_identity(nc, ident)
    # constant ones/pad columns of vtk (memset can't write float32r directly)
    onepad = pers.tile([128, 2], F32, tag="onepad")
    nc.vector.memset(onepad[:, 0:1], 1.0)
    nc.vector.memset(onepad[:, 1:2], 0.0)
    onepad_b = bass.AP(
        tensor=onepad.tensor,
        offset=onepad.offset,
        ap=[onepad.ap[0], [0, B * NKT], onepad.ap[1]],
    )
    for h in range(HPC):
        nc.vector.tensor_copy(
            vtk[:, :, h * HW + DH : h * HW + DH + 2], onepad_b
        )

    # Phase A: Q/K/V projections in o-major layout, then transpose V to
    # token-major k-tiles.
    with (
        tc.tile_pool(name="psA", bufs=2, space="PSUM") as psA,
        tc.tile_pool(name="xin", bufs=8) as xin,
    ):
        for t in range(NTCH):
            ps_q = psA.tile([128, TCH], F32, tag="ps_q")
            ps_k = psA.tile([128, TCH], F32, tag="ps_k")
            ps_v = psA.tile([128, TCH], F32, tag="ps_v")
            for i in range(NI):
                xt = xin.tile([128, TCH], MM_DT, tag="xt")
                nc.sync.dma_start(xt, xT[i, :, t * TCH : (t + 1) * TCH])
                st, sp = (i == 0), (i == NI - 1)
                nc.tensor.matmul(ps_q, wq_sb[:, i, :], xt, start=st, stop=sp)
                nc.tensor.matmul(ps_k, wk_sb[:, i, :], xt, start=st, stop=sp)
                nc.tensor.matmul(ps_v, wv_sb[:, i, :], xt, start=st, stop=sp)
            sl = slice(t * TCH, (t + 1) * TCH)
            nc.vector.tensor_scalar_add(qT[:, sl], ps_q, bq_sb)
            nc.vector.tensor_scalar_add(kT[:, sl], ps_k, bk_sb)
            nc.vector.tensor_scalar_add(vT[:, sl], ps_v, bv_sb)
            # transpose this chunk's V to token-major right away (keeps the
            # PE busy across the phase boundary)
            for g in range(t * TCH // 128, (t + 1) * TCH // 128):
                ps_t = psA.tile([128, 128], VT_DT, tag="ps_t")
                nc.tensor.transpose(ps_t, vT[:, g * 128 : (g + 1) * 128], ident)
                for h in range(HPC):
                    nc.vector.tensor_copy(
                        vtk[:, g, h * HW : h * HW + DH],
                        ps_t[:, h * DH : (h + 1) * DH],
                    )

    # Phase B: attention + output projection.
    with (
        tc.tile_pool(name="psB", bufs=2, space="PSUM") as psB,
        tc.tile_pool(name="att", bufs=5) as att,
        tc.tile_pool(name="sm", bufs=3) as sm,
        tc.tile_pool(name="yo_p", bufs=3) as yo_p,
    ):
        for b in range(B):
            for qc in range(NQCH):
                q0 = b * S + qc * QCH
                ctx_sb = sm.tile([128, QCH], MM_DT, tag="ctx")
                # Scores for BOTH heads interleaved: head h uses SBUF
                # partitions [h*64,(h+1)*64) so the two matmuls land on
                # different row-tiles of the 64x128-tiled PE array and stream
                # concurrently. One exp covers both heads' psum banks.
                at_tiles = []
                for kt in range(NKT):
                    g = b * NKT + kt
                    ps_s = psB.tile([128, 2, QCH], F32, tag="ps_s")
                    for h in range(HPC):
                        hs = slice(h * DH, (h + 1) * DH)
                        nc.tensor.matmul(
                            ps_s[:, h, :],
                            kT[hs, g * 128 : (g + 1) * 128],
                            qT[hs, q0 : q0 + QCH],
                            start=True,
                            stop=True,
                        )
                    at = att.tile([128, 2, QCH], MM_DT, tag="at")
                    nc.scalar.activation(at, ps_s, EXP, scale=0.125)
                    at_tiles.append(at)
                pvs = [
                    psB.tile([HW, QCH], F32, tag="ps_pv", name=f"pv{h}")
                    for h in range(HPC)
                ]
                for kt in range(NKT):
                    g = b * NKT + kt
                    for h in range(HPC):
                        nc.tensor.matmul(
                            pvs[h],
                            vtk[:, g, h * HW : (h + 1) * HW],
                            at_tiles[kt][:, h, :],
                            start=(kt == 0),
                            stop=(kt == NKT - 1),
                        )
                for h in range(HPC):
                    # normalize: ctx rows for this head = pv[0:64] * recip(pv[64])
                    rraw = sm.tile([1, QCH], F32, tag="rraw")
                    nc.vector.tensor_copy(rraw, pvs[h][DH : DH + 1, :])
                    rrow = sm.tile([1, QCH], F32, tag="rrow")
                    nc.vector.reciprocal_approx_fast(rrow, rraw)
                    nrm = sm.tile([DH, QCH], F32, tag="nrm")
                    nc.gpsimd.partition_broadcast(nrm, rrow)
                    nc.vector.tensor_mul(
                        ctx_sb[h * DH : (h + 1) * DH, :], pvs[h][0:DH, :], nrm
                    )
                for t4 in range(QCH // 128):
                    yo = yo_p.tile([128, D], F32, tag="yo")
                    for nch in range(D // 512):
                        ps_o = psB.tile([128, 512], F32, tag="ps_o")
                        nc.tensor.matmul(
                            ps_o,
                            ctx_sb[:, t4 * 128 : (t4 + 1) * 128],
                            woT_sb[:, nch * 512 : (nch + 1) * 512],
                            start=True,
                            stop=True,
                        )
                        nc.vector.tensor_copy(yo[:, nch * 512 : (nch + 1) * 512], ps_o)
                    r0 = q0 + t4 * 128
                    nc.sync.dma_start(y[r0 : r0 + 128, :], yo)


_NC_CACHE = {}


def _build_nc(repeats=1):
    if repeats in _NC_CACHE:
        return _NC_CACHE[repeats]
    nc = bacc.Bacc("TRN2", target_bir_lowering=False, debug=False, num_devices=N_CORES)
    xT = nc.dram_tensor("xT", [NI, 128, T], MM_DT, kind="ExternalInput").ap()
    wq = nc.dram_tensor("wq", [128, NI, OPC], MM_DT, kind="ExternalInput").ap()
    wk = nc.dram_tensor("wk", [128, NI, OPC], MM_DT, kind="ExternalInput").ap()
    wv = nc.dram_tensor("wv", [128, NI, OPC], MM_DT, kind="ExternalInput").ap()
    woT = nc.dram_tensor("woT", [128, D], MM_DT, kind="ExternalInput").ap()
    bq = nc.dram_tensor("bq", [128, 1], F32, kind="ExternalInput").ap()
    bk = nc.dram_tensor("bk", [128, 1], F32, kind="ExternalInput").ap()
    bv = nc.dram_tensor("bv", [128, 1], F32, kind="ExternalInput").ap()
    y = nc.dram_tensor("y", [T, D], F32, kind="ExternalOutput").ap()
    with tile.TileContext(nc) as tc:
        for _ in range(repeats):
            _mha_kernel(tc, y, xT, wq, wk, wv, woT, bq, bk, bv)
    nc.compile()
    _NC_CACHE[repeats] = nc
    return nc


def _prep_in_maps(inputs):
    x = np.asarray(inputs["x"], np.float32)
    Wq = np.asarray(inputs["Wq"], np.float32)
    Wk = np.asarray(inputs["Wk"], np.float32)
    Wv = np.asarray(inputs["Wv"], np.float32)
    Wo = np.asarray(inputs["Wo"], np.float32)
    bq = np.asarray(inputs["bq"], np.float32)
    bk = np.asarray(inputs["bk"], np.float32)
    bv = np.asarray(inputs["bv"], np.float32)

    xT_np = np.ascontiguousarray(x.reshape(T, D).T).reshape(NI, 128, T).astype(MM_NP)

    def _w_slice(W, c):
        # [128(p), NI, OPC]: [p, i, o] = W[c*OPC+o, i*128+p]
        A = np.ascontiguousarray(W[c * OPC : (c + 1) * OPC, :].T)  # [D, OPC]
        return np.ascontiguousarray(A.reshape(NI, 128, OPC).transpose(1, 0, 2)).astype(
            MM_NP
        )

    in_maps = []
    for c in range(N_CORES):
        sl = slice(c * OPC, (c + 1) * OPC)
        in_maps.append(
            {
                "xT": xT_np,
                "wq": _w_slice(Wq, c),
                "wk": _w_slice(Wk, c),
                "wv": _w_slice(Wv, c),
                "woT": np.ascontiguousarray(Wo[:, sl].T).astype(MM_NP),
                "bq": bq[sl].reshape(OPC, 1).copy(),
                "bk": bk[sl].reshape(OPC, 1).copy(),
                "bv": bv[sl].reshape(OPC, 1).copy(),
            }
        )
    return in_maps


def kernel(**inputs) -> np.ndarray:
    nc = _build_nc()
    in_maps = _prep_in_maps(inputs)
    res = run_bass_kernel_spmd(nc, in_maps, core_ids=list(range(N_CORES)))
    bo = np.asarray(inputs["bo"], np.float32)
    y = np.zeros((T, D), np.float64)
    for c in range(N_CORES):
        y += res.results[c]["y"].astype(np.float64)
    y = (y + bo).astype(np.float32)
    return y.reshape(B, S, D)


# revision 30
# speedup vs baseline: 31.2333x; 1.0145x over previous
"""Multi-head self-attention (B=2, S=2048, D=1024, H=16) on 8 Trainium2 cores.

Sharding: Megatron-style tensor parallelism on the head dimension.
Each core owns 2 heads (128 of the 1024 model dims):
  - Wq/Wk/Wv column-sharded: core c computes Q/K/V for dims [c*128,(c+1)*128)
  - attention for its 2 heads over both batches
  - Wo row-sharded: core c produces a partial output [4096, 1024]
  - host sums the 8 partials and adds bo.

Per-core device layouts (matmuls run as float32r = full-rate PE with
~tf32 multiply precision, fp32 accumulate):
  qT/kT: [128(out-dim), 4096(token)]  "o-major"
  v:     token-major k-tiles [128(token), 132] = 2x [head(64) | ones | pad]
         (the ones column makes the PV matmul also produce the softmax
          normalizer as output row 64; pad keeps the stationary free dim even,
          a float32r requirement)
  scores are computed transposed: sT[k, q] = (kT tile).T @ qT chunk, so the
  softmax sum reduces over the PARTITION dim -- done for free by the ones row
  in the PV matmul instead of a vector reduction. exp() needs no max
  subtraction: scores*0.125 are ~N(0,1) for this problem family, far from
  fp32 overflow.
"""

import os
import numpy as np
import ml_dtypes
from contextlib import ExitStack

import concourse.bass as bass
import concourse.tile as tile
from concourse import bacc, mybir
from concourse.bass_utils import run_bass_kernel_spmd
from concourse.masks import make_identity

B, S, D = 2, 2048, 1024
H, DH = 16, 64
T = B * S                  # 4096 tokens total
N_CORES = 8
OPC = D // N_CORES         # 128 out dims per core
HPC = H // N_CORES         # 2 heads per core
NI = D // 128              # 8 contraction chunks of 128
TCH = 512                  # projection token chunk
NTCH = T // TCH            # 8
QCH = 512                  # attention q chunk
NQCH = S // QCH            # 4 per batch
NKT = S // 128             # 16 key tiles per batch
HW = DH + 2                # 66 cols per head in the v tile (data|ones|pad)
VW = HPC * HW              # 132

F32 = mybir.dt.float32
F32R = mybir.dt.float32r
BF16 = mybir.dt.bfloat16
EXP = mybir.ActivationFunctionType.Exp

# matmul operand dtype: "f32r" (~tf32 precision, 2 PE cycles/row) or
# "bf16" (1 PE cycle/row + fast weight load, ~bf16 precision)
MM_MODE = os.environ.get("MHA_MM_DT", "f32r")
if MM_MODE == "bf16":
    MM_DT, MM_NP = BF16, ml_dtypes.bfloat16
else:
    MM_DT, MM_NP = F32R, np.float32
# dtype of the pre-transpose V staging and the transpose identity
VT_DT = BF16 if MM_MODE == "bf16" else F32


def _mha_kernel(tc, y, xT, wq, wk, wv, woT, bq, bk, bv):
    with ExitStack() as ctx:
        _mha_kernel_inner(ctx, tc, y, xT, wq, wk, wv, woT, bq, bk, bv)


def _mha_kernel_inner(ctx: ExitStack, tc, y, xT, wq, wk, wv, woT, bq, bk, bv):
    nc = tc.nc
    pers = ctx.enter_context(tc.tile_pool(name="pers", bufs=1))

    qT = pers.tile([128, T], MM_DT, tag="qT")
    kT = pers.tile([128, T], MM_DT, tag="kT")
    vT = pers.tile([128, T], VT_DT, tag="vT")
    vtk = pers.tile([128, B * NKT, VW], MM_DT, tag="vtk")
    wq_sb = pers.tile([128, NI, OPC], MM_DT, tag="wq")
    wk_sb = pers.tile([128, NI, OPC], MM_DT, tag="wk")
    wv_sb = pers.tile([128, NI, OPC], MM_DT, tag="wv")
    woT_sb = pers.tile([128, D], MM_DT, tag="wo")
    bq_sb = pers.tile([128, 1], F32, tag="bq")
    bk_sb = pers.tile([128, 1], F32, tag="bk")
    bv_sb = pers.tile([128, 1], F32, tag="bv")
    ident = pers.tile([128, 128], VT_DT, tag="ident")

    # weights on the gpsimd DMA queue so the sync queue starts streaming x
    # tiles immediately
    nc.gpsimd.dma_start(wq_sb, wq)
    nc.gpsimd.dma_start(wk_sb, wk)
    nc.gpsimd.dma_start(wv_sb, wv)
    nc.gpsimd.dma_start(woT_sb, woT)
    nc.gpsimd.dma_start(bq_sb, bq)
    nc.gpsimd.dma_start(bk_sb, bk)
    nc.gpsimd.dma_start(bv_sb, bv)
    make_identity(nc, ident)
    # constant ones/pad columns of vtk (memset can't write float32r directly)
    onepad = pers.tile([128, 2], F32, tag="onepad")
    nc.vector.memset(onepad[:, 0:1], 1.0)
    nc.vector.memset(onepad[:, 1:2], 0.0)
    onepad_b = bass.AP(
        tensor=onepad.tensor,
        offset=onepad.offset,
        ap=[onepad.ap[0], [0, B * NKT], onepad.ap[1]],
    )
    for h in range(HPC):
        nc.vector.tensor_copy(
            vtk[:, :, h * HW + DH : h * HW + DH + 2], onepad_b
        )

    # Phase A: Q/K/V projections in o-major layout, then transpose V to
    # token-major k-tiles.
    with (
        tc.tile_pool(name="psA", bufs=2, space="PSUM") as psA,
        tc.tile_pool(name="xin", bufs=8) as xin,
    ):
        for t in range(NTCH):
            ps_q = psA.tile([128, TCH], F32, tag="ps_q")
            ps_k = psA.tile([128, TCH], F32, tag="ps_k")
            ps_v = psA.tile([128, TCH], F32, tag="ps_v")
            for i in range(NI):
                xt = xin.tile([128, TCH], MM_DT, tag="xt")
                nc.sync.dma_start(xt, xT[i, :, t * TCH : (t + 1) * TCH])
                st, sp = (i == 0), (i == NI - 1)
                nc.tensor.matmul(ps_q, wq_sb[:, i, :], xt, start=st, stop=sp)
                nc.tensor.matmul(ps_k, wk_sb[:, i, :], xt, start=st, stop=sp)
                nc.tensor.matmul(ps_v, wv_sb[:, i, :], xt, start=st, stop=sp)
            sl = slice(t * TCH, (t + 1) * TCH)
            nc.vector.tensor_scalar_add(qT[:, sl], ps_q, bq_sb)
            nc.vector.tensor_scalar_add(kT[:, sl], ps_k, bk_sb)
            nc.vector.tensor_scalar_add(vT[:, sl], ps_v, bv_sb)
            # transpose this chunk's V to token-major right away (keeps the
            # PE busy across the phase boundary)
            for g in range(t * TCH // 128, (t + 1) * TCH // 128):
                ps_t = psA.tile([128, 128], VT_DT, tag="ps_t")
                nc.tensor.transpose(ps_t, vT[:, g * 128 : (g + 1) * 128], ident)
                for h in range(HPC):
                    nc.vector.tensor_copy(
                        vtk[:, g, h * HW : h * HW + DH],
                        ps_t[:, h * DH : (h + 1) * DH],
                    )

    # Phase B: attention + output projection.
    with (
        tc.tile_pool(name="psB", bufs=2, space="PSUM") as psB,
        tc.tile_pool(name="att", bufs=5) as att,
        tc.tile_pool(name="sm", bufs=3) as sm,
        tc.tile_pool(name="yo_p", bufs=3) as yo_p,
    ):
        for b in range(B):
            for qc in range(NQCH):
                q0 = b * S + qc * QCH
                ctx_sb = sm.tile([128, QCH], MM_DT, tag="ctx")
                # Scores for BOTH heads interleaved: head h uses SBUF
                # partitions [h*64,(h+1)*64) so the two matmuls land on
                # different row-tiles of the 64x128-tiled PE array and stream
                # concurrently. One exp covers both heads' psum banks.
                at_tiles = []
                for kt in range(NKT):
                    g = b * NKT + kt
                    ps_s = psB.tile([128, 2, QCH], F32, tag="ps_s")
                    for h in range(HPC):
                        hs = slice(h * DH, (h + 1) * DH)
                        nc.tensor.matmul(
                            ps_s[:, h, :],
                            kT[hs, g * 128 : (g + 1) * 128],
                            qT[hs, q0 : q0 + QCH],
                            start=True,
                            stop=True,
                        )
                    at = att.tile([128, 2, QCH], MM_DT, tag="at")
                    nc.scalar.activation(at, ps_s, EXP, scale=0.125)
                    at_tiles.append(at)
                pvs = [
                    psB.tile([HW, QCH], F32, tag="ps_pv", name=f"pv{h}")
                    for h in range(HPC)
                ]
                for kt in range(NKT):
                    g = b * NKT + kt
                    for h in range(HPC):
                        nc.tensor.matmul(
                            pvs[h],
                            vtk[:, g, h * HW : (h + 1) * HW],
                            at_tiles[kt][:, h, :],
                            start=(kt == 0),
                            stop=(kt == NKT - 1),
                        )
                for h in range(HPC):
                    # normalize: ctx rows for this head = pv[0:64] * recip(pv[64])
                    rraw = sm.tile([1, QCH], F32, tag="rraw")
                    nc.vector.tensor_copy(rraw, pvs[h][DH : DH + 1, :])
                    rrow = sm.tile([1, QCH], F32, tag="rrow")
                    nc.vector.reciprocal_approx_fast(rrow, rraw)
                    nrm = sm.tile([DH, QCH], F32, tag="nrm")
                    nc.gpsimd.partition_broadcast(nrm, rrow)
                    nc.vector.tensor_mul(
                        ctx_sb[h * DH : (h + 1) * DH, :], pvs[h][0:DH, :], nrm
                    )
                for t4 in range(QCH // 128):
                    yo = yo_p.tile([128, D], F32, tag="yo")
                    for nch in range(D // 512):
                        ps_o = psB.tile([128, 512], F32, tag="ps_o")
                        nc.tensor.matmul(
                            ps_o,
                            ctx_sb[:, t4 * 128 : (t4 + 1) * 128],
                            woT_sb[:, nch * 512 : (nch + 1) * 512],
                            start=True,
                            stop=True,
                        )
                        nc.vector.tensor_copy(yo[:, nch * 512 : (nch + 1) * 512], ps_o)
                    r0 = q0 + t4 * 128
                    nc.sync.dma_start(y[r0 : r0 + 128, :], yo)


_NC_CACHE = {}


def _build_nc(repeats=1):
    if repeats in _NC_CACHE:
        return _NC_CACHE[repeats]
    nc = bacc.Bacc("TRN2", target_bir_lowering=False, debug=False, num_devices=N_CORES)
    xT = nc.dram_tensor("xT", [NI, 128, T], MM_DT, kind="ExternalInput").ap()
    wq = nc.dram_tensor("wq", [128, NI, OPC], MM_DT, kind="ExternalInput").ap()
    wk = nc.dram_tensor("wk", [128, NI, OPC], MM_DT, kind="ExternalInput").ap()
    wv = nc.dram_tensor("wv", [128, NI, OPC], MM_DT, kind="ExternalInput").ap()
    woT = nc.dram_tensor("woT", [128, D], MM_DT, kind="ExternalInput").ap()
    bq = nc.dram_tensor("bq", [128, 1], F32, kind="ExternalInput").ap()
    bk = nc.dram_tensor("bk", [128, 1], F32, kind="ExternalInput").ap()
    bv = nc.dram_tensor("bv", [128, 1], F32, kind="ExternalInput").ap()
    y = nc.dram_tensor("y", [T, D], F32, kind="ExternalOutput").ap()
    with tile.TileContext(nc) as tc:
        for _ in range(repeats):
            _mha_kernel(tc, y, xT, wq, wk, wv, woT, bq, bk, bv)
    nc.compile()
    _NC_CACHE[repeats] = nc
    return nc


def _prep_in_maps(inputs):
    x = np.asarray(inputs["x"], np.float32)
    Wq = np.asarray(inputs["Wq"], np.float32)
    Wk = np.asarray(inputs["Wk"], np.float32)
    Wv = np.asarray(inputs["Wv"], np.float32)
    Wo = np.asarray(inputs["Wo"], np.float32)
    bq = np.asarray(inputs["bq"], np.float32)
    bk = np.asarray(inputs["bk"], np.float32)
    bv = np.asarray(inputs["bv"], np.float32)

    xT_np = np.ascontiguousarray(x.reshape(T, D).T).reshape(NI, 128, T).astype(MM_NP)

    def _w_slice(W, c):
        # [128(p), NI, OPC]: [p, i, o] = W[c*OPC+o, i*128+p]
        A = np.ascontiguousarray(W[c * OPC : (c + 1) * OPC, :].T)  # [D, OPC]
        return np.ascontiguousarray(A.reshape(NI, 128, OPC).transpose(1, 0, 2)).astype(
            MM_NP
        )

    in_maps = []
    for c in range(N_CORES):
        sl = slice(c * OPC, (c + 1) * OPC)
        in_maps.append(
            {
                "xT": xT_np,
                "wq": _w_slice(Wq, c),
                "wk": _w_slice(Wk, c),
                "wv": _w_slice(Wv, c),
                "woT": np.ascontiguousarray(Wo[:, sl].T).astype(MM_NP),
                "bq": bq[sl].reshape(OPC, 1).copy(),
                "bk": bk[sl].reshape(OPC, 1).copy(),
                "bv": bv[sl].reshape(OPC, 1).copy(),
            }
        )
    return in_maps


def kernel(**inputs) -> np.ndarray:
    nc = _build_nc()
    in_maps = _prep_in_maps(inputs)
    res = run_bass_kernel_spmd(nc, in_maps, core_ids=list(range(N_CORES)))
    bo = np.asarray(inputs["bo"], np.float32)
    y = np.zeros((T, D), np.float64)
    for c in range(N_CORES):
        y += res.results[c]["y"].astype(np.float64)
    y = (y + bo).astype(np.float32)
    return y.reshape(B, S, D)
